# revision 3
# baseline (speedup 1.0000x reference)
"""AFT2D attention Trainium2 kernel (8 NeuronCores, data-parallel over batch).

Math: the reference's 5x5 windowed attention with positional bias
    wgt = exp(w_h[ii]*(di-h) + w_v[jj]*(dj-w) + k[h+di, w+dj]) * mask
factorizes exactly: exp(bias) separates into per-row and per-column factors,
so with ek = exp(k), u = ek*v, s = sum_d ek:
    out  = A @ (B ∘w u)      (two banded 64x64 matrix passes, h then w)
    norm = A @ (B ∘w s)
    y    = (out @ Wp^T) / (norm + eps)      (normalize commutes past Wp)
where A[h,h'] = exp(w_h[h'-h+R]*((h'-h)-h)) on the band, B likewise over w.

Per-core pipeline (b_loc=2 images, partitions = (b,w) then (b,h)):
  1. k|v matmul: x-tiles (pre-transposed on host, bf16) as PE stationary,
     streaming [Wk^T | Wv^T]; out psum [128=(b,w), 512] per h.
  2. ACT exp (accum_out -> s), DVE u = ek * v  -> u slab [(b,w), (h,d)] bf16.
  3. Horizontal pass: one matmul per 512-chunk with stationary
     blkdiag(B^T,B^T); drains -> T slab.
  4. h<->w shuffle via per-(b,h) gather DMAs -> Tt slab [(b,h), (w,d)].
  5. Vertical pass with swapped operands (lhsT = Tt chunk, rhs =
     blkdiag(A^T,A^T)) so psum comes out feature-major [d-half, (b,h)] --
     exactly the projection's lhsT; no extra transpose.
  6. Projection matmul + drain scaled by 1/norm (per-partition scale), y bf16.
Norm path runs in f32 on the side (tiny 64x64 matmuls).
"""
import sys

sys.path.insert(0, "/opt/trn_rl_repo")

import numpy as np
import ml_dtypes

import concourse.bass as bass
import concourse.mybir as mybir
import concourse.tile as tile
from concourse.bass_utils import run_bass_kernel_spmd

bf16 = ml_dtypes.bfloat16

N_CORES = 8
B_FULL, H, W, C = 16, 64, 64, 256
D = 256   # HID
O = 256   # OUT
R = 2
WS = 2 * R + 1
B_LOC = B_FULL // N_CORES  # 2

LAST_RESULT = None
_CACHED_NC = None


def _split_multi_waits(nc, max_waits=1):
    """This container's walrus accepts at most ONE sync-wait per instruction;
    hoist extras into standalone same-engine no-ops (order-preserving)."""
    n_new = 0
    for func in nc.m.functions:
        for blk in func.blocks:
            new_insts = []
            for inst in blk.instructions:
                si = inst.sync_info
                if si is not None and len(si.on_wait) > max_waits:
                    waits = list(si.on_wait)
                    for w in waits[:-max_waits]:
                        nop = mybir.InstNoOp(
                            name=f"waitsplit-{n_new}-{inst.name}", ins=[], outs=[])
                        nop.engine = inst.engine
                        nop.sync_info = mybir.SyncInfo(on_wait=[w], on_update=[])
                        new_insts.append(nop)
                        n_new += 1
                    si.on_wait = waits[-max_waits:]
                new_insts.append(inst)
            blk.instructions = new_insts
    return n_new


def _build_nc():
    fp32 = mybir.dt.float32
    bft = mybir.dt.bfloat16

    nc = bass.Bass()
    xt_ext = nc.declare_dram_parameter("xt", [H, 128, 2, 128], bft, isOutput=False)
    wkv_ext = nc.declare_dram_parameter("wkv", [128, 2, 2 * D], bft, isOutput=False)
    wp_ext = nc.declare_dram_parameter("wp", [128, 2, O], bft, isOutput=False)
    bh_ext = nc.declare_dram_parameter("bh_bd", [128, 128], bft, isOutput=False)
    av_ext = nc.declare_dram_parameter("av_bd", [128, 128], bft, isOutput=False)
    bhf_ext = nc.declare_dram_parameter("bh_f32", [128, 128], fp32, isOutput=False)
    a64_ext = nc.declare_dram_parameter("a64t_f32", [64, 64], fp32, isOutput=False)
    y_ext = nc.declare_dram_parameter("y", [B_LOC, H, W, O], bft, isOutput=True)

    with tile.TileContext(nc) as tc:
        with (
            tc.tile_pool(name="const", bufs=1) as cpool,
            tc.tile_pool(name="xt", bufs=4) as xt_pool,
            tc.tile_pool(name="ek", bufs=4) as ek_pool,
            tc.tile_pool(name="slab", bufs=1) as slab_pool,
            tc.tile_pool(name="ot", bufs=4) as ot_pool,
            tc.tile_pool(name="y", bufs=4) as y_pool,
            tc.tile_pool(name="ps_kv", bufs=2, space="PSUM") as ps_kv_pool,
            tc.tile_pool(name="ps_t", bufs=2, space="PSUM") as ps_t_pool,
            tc.tile_pool(name="ps_g", bufs=2, space="PSUM") as ps_g_pool,
            tc.tile_pool(name="ps_y", bufs=2, space="PSUM") as ps_y_pool,
        ):
            # ---- constants
            wkv_sb = cpool.tile([128, 2, 2 * D], bft)
            nc.sync.dma_start(out=wkv_sb[:], in_=wkv_ext[:])
            wp_sb = cpool.tile([128, 2, O], bft)
            nc.sync.dma_start(out=wp_sb[:], in_=wp_ext[:])
            bh_bd = cpool.tile([128, 128], bft)
            nc.sync.dma_start(out=bh_bd[:], in_=bh_ext[:])
            av_bd = cpool.tile([128, 128], bft)
            nc.sync.dma_start(out=av_bd[:], in_=av_ext[:])
            bh_f32 = cpool.tile([128, 128], fp32)
            nc.sync.dma_start(out=bh_f32[:], in_=bhf_ext[:])
            a64t = cpool.tile([64, 64], fp32)
            nc.sync.dma_start(out=a64t[:], in_=a64_ext[:])

            u_slab = slab_pool.tile([128, H, D], bft)      # [(b,w), h, d]
            t_slab = slab_pool.tile([128, H, D], bft)      # [(b,w), h, d]
            tt_slab = slab_pool.tile([128, W, D], bft)     # [(b,h), w, d]
            s_slab = slab_pool.tile([128, H], fp32)        # [(b,w), h]
            u_flat = u_slab.rearrange("p h d -> p (h d)")
            t_flat = t_slab.rearrange("p h d -> p (h d)")
            tt_flat = tt_slab.rearrange("p w d -> p (w d)")

            # ---- phase 1: k|v matmuls + exp + u
            for h in range(H):
                xt = xt_pool.tile([128, 2, 128], bft)
                nc.sync.dma_start(out=xt[:], in_=xt_ext[h])
                ps_kv = ps_kv_pool.tile([128, 2 * D], fp32)
                nc.tensor.matmul(ps_kv[:], xt[:, 0, :], wkv_sb[:, 0, :],
                                 start=True, stop=False)
                nc.tensor.matmul(ps_kv[:], xt[:, 1, :], wkv_sb[:, 1, :],
                                 start=False, stop=True)
                ek = ek_pool.tile([128, D], bft)
                nc.scalar.activation(ek[:], ps_kv[:, 0:D],
                                     mybir.ActivationFunctionType.Exp,
                                     accum_out=s_slab[:, h:h + 1])
                nc.vector.tensor_mul(u_slab[:, h, :], ek[:], ps_kv[:, D:2 * D])

            # ---- phase 3: horizontal pass (contract w' on partitions)
            n_hchunks = H * D // 512
            for c in range(n_hchunks):
                ps_t = ps_t_pool.tile([128, 512], fp32)
                nc.tensor.matmul(ps_t[:], bh_bd[:], u_flat[:, c * 512:(c + 1) * 512],
                                 start=True, stop=True)
                if c % 2 == 0:
                    nc.scalar.copy(t_flat[:, c * 512:(c + 1) * 512], ps_t[:])
                else:
                    nc.vector.tensor_copy(t_flat[:, c * 512:(c + 1) * 512], ps_t[:])

            # norm-horizontal (swapped: out comes transposed [h, (b,w)])
            ps_sh = ps_t_pool.tile([64, 128], fp32, tag="ps_t")
            nc.tensor.matmul(ps_sh[:], s_slab[:], bh_f32[:], start=True, stop=True)
            sth = cpool.tile([64, 128], fp32, tag="sth")
            nc.vector.tensor_copy(sth[:], ps_sh[:])

            # norm-vertical: per-b matmuls into partition halves via col tiling
            ps_n = ps_y_pool.tile([128, 64], fp32, tag="ps_y")
            for b in range(B_LOC):
                nc.tensor.matmul(ps_n[b * 64:(b + 1) * 64, :], a64t[:],
                                 sth[:, b * 64:(b + 1) * 64],
                                 start=True, stop=True,
                                 tile_position=(0, b * 64))
            ntmp = cpool.tile([128, 64], fp32, tag="ntmp")
            nc.vector.tensor_scalar_add(ntmp[:], ps_n[:], 1e-8)
            rnorm = cpool.tile([128, 64], fp32, tag="rnorm")
            nc.vector.reciprocal(rnorm[:], ntmp[:])

            # ---- phase 4: h<->w shuffle (gather per output row)
            t_v = t_slab.rearrange("(b w) h d -> b w h d", b=B_LOC)
            for h in range(H):
                for b in range(B_LOC):
                    p = b * 64 + h
                    nc.sync.dma_start(out=tt_flat[p:p + 1, :], in_=t_v[b, :, h, :])

            # ---- phase 5+6: vertical pass (swapped -> feature-major) + proj
            for g in range(W // 2):           # w-pair per psum group
                ps_g = ps_g_pool.tile([128, 512], fp32)
                for q in range(4):
                    cch = g * 4 + q           # chunk = (w = cch>>1, dhalf = cch&1)
                    nc.tensor.matmul(ps_g[:, q * 128:(q + 1) * 128],
                                     tt_flat[:, cch * 128:(cch + 1) * 128],
                                     av_bd[:], start=True, stop=True)
                ot = ot_pool.tile([128, 512], bft)
                if g % 2 == 0:
                    nc.scalar.copy(ot[:], ps_g[:])
                else:
                    nc.vector.tensor_copy(ot[:], ps_g[:])

                for wi in range(2):
                    w = 2 * g + wi
                    base = wi * 256
                    ps_y = ps_y_pool.tile([128, O], fp32)
                    nc.tensor.matmul(ps_y[:], ot[:, base:base + 128],
                                     wp_sb[:, 0, :], start=True, stop=False)
                    nc.tensor.matmul(ps_y[:], ot[:, base + 128:base + 256],
                                     wp_sb[:, 1, :], start=False, stop=True)
                    yt = y_pool.tile([128, O], bft)
                    if w % 2 == 0:
                        nc.scalar.mul(yt[:], ps_y[:], rnorm[:, w:w + 1])
                    else:
                        nc.vector.tensor_scalar_mul(yt[:], ps_y[:], rnorm[:, w:w + 1])
                    nc.sync.dma_start(
                        out=y_ext[:, :, w, :].rearrange("b h o -> (b h) o"),
                        in_=yt[:])

    _split_multi_waits(nc)
    return nc


def _host_prep(x, w_h, w_v, Wk, Wv, Wp):
    """Build per-core input maps (all layout/packing on host, compute on device)."""
    A = np.zeros((H, H), np.float32)
    Bm = np.zeros((W, W), np.float32)
    for h in range(H):
        for hp in range(max(0, h - R), min(H, h + R + 1)):
            A[h, hp] = np.exp(w_h[hp - h + R] * ((hp - h) - h))
    for w in range(W):
        for wp in range(max(0, w - R), min(W, w + R + 1)):
            Bm[w, wp] = np.exp(w_v[wp - w + R] * ((wp - w) - w))

    eye2 = np.eye(2, dtype=np.float32)
    bh_bd = np.kron(eye2, Bm.T).astype(bf16)          # lhsT for horizontal
    av_bd = np.kron(eye2, A.T).astype(bf16)           # rhs for swapped vertical
    bh_f32 = np.kron(eye2, Bm.T).astype(np.float32)
    a64t_f32 = A.T.astype(np.float32)

    # wkv[ci, cc, j] = Wk[j, cc*128+ci] (j<256) else Wv[j-256, ...]
    wkv = np.concatenate([Wk.T, Wv.T], axis=1)        # [C, 2D] = [c, (k|v)]
    wkv = wkv.reshape(2, 128, 2 * D).transpose(1, 0, 2).astype(bf16)
    wp = Wp.T.reshape(2, 128, O).transpose(1, 0, 2).astype(bf16)  # [di, dc, o]

    in_maps = []
    for c in range(N_CORES):
        xl = x[c * B_LOC:(c + 1) * B_LOC]             # (2, 64, 64, 256)
        t = xl.reshape(B_LOC, H, W, 2, 128)           # b h w cc ci
        xt = np.ascontiguousarray(
            t.transpose(1, 4, 3, 0, 2).reshape(H, 128, 2, 128)).astype(bf16)
        in_maps.append({
            "xt": xt, "wkv": wkv, "wp": wp,
            "bh_bd": bh_bd, "av_bd": av_bd,
            "bh_f32": bh_f32, "a64t_f32": a64t_f32,
        })
    return in_maps


def kernel(x, w_h, w_v, Wk, Wv, Wp):
    global LAST_RESULT, _CACHED_NC
    x = np.asarray(x, dtype=np.float32)
    w_h = np.asarray(w_h, dtype=np.float32)
    w_v = np.asarray(w_v, dtype=np.float32)
    Wk = np.asarray(Wk, dtype=np.float32)
    Wv = np.asarray(Wv, dtype=np.float32)
    Wp = np.asarray(Wp, dtype=np.float32)

    in_maps = _host_prep(x, w_h, w_v, Wk, Wv, Wp)
    if _CACHED_NC is None:
        _CACHED_NC = _build_nc()
    res = run_bass_kernel_spmd(_CACHED_NC, in_maps, core_ids=list(range(N_CORES)))
    LAST_RESULT = res

    out = np.empty((B_FULL, H, W, O), np.float32)
    for c in range(N_CORES):
        yc = np.asarray(res.results[c]["y"]).astype(np.float32)
        out[c * B_LOC:(c + 1) * B_LOC] = yc
    return out
